# revision 12
# baseline (speedup 1.0000x reference)
"""Causal attention (B=1, H=16, S=4096, D=64, f32) on 8 trn2 NeuronCores.

Strategy (head-parallel, 2 heads per core), v2:
  - Host pre-transposes Q, K per head to [D, S] fp16 (d-major) so QK^T
    needs no on-device transpose: S^T[k, q] = sum_d K^T[d,k] Q^T[d,q].
    One fp16 matmul per (k-tile, q-block): lhsT [64,128], rhs [64,512],
    ~217 ns (output-partition bound: 128 scores/cycle is the PE limit).
  - Causal masking is additive and done on the PE: tiny fp16 matmuls
    (~60 ns) accumulate -480 into masked score regions of PSUM (triangle
    / full-block lhsT against an identity rhs), so exp(0.125*s) is ~1e-26
    there and no vector-engine masking is needed.
  - exp is split across two engines, processing chunks of 2 k-tiles:
      * ScalarE ACTIVATE: exact exp, fp8e4 out with an e^-2.5 shift
        (keeps exp under fp8e4m3 max 448); consumed by fp8
        DoubleRow PV matmuls - 2 k-tiles per 512-cycle stream (ATTN_DR=1),
        or fp16 out + fp16 PV with ATTN_DR=0.
      * VectorE tensor_scalar: Schraudolph bit-trick exp: int16
        rint(A*x+B) whose bits read as fp16 give e^(0.125x-c0) to ~3%;
        consumed (bitcast) by fp16 PV matmuls. The f32->int16 conversion
        saturates, so -480-masked scores become -32768 = fp16 -0.0.
    Diagonal chunks always go to ScalarE: they hold the dominant
    self-attention weights (s(q,q) ~ +8 sigma), where 3% error is not
    acceptable.
  - PV: O^T[d, q] = sum_k V[k, d] P^T[k, q], PSUM-accumulated over
    k-tiles. l[q] = sum_k exp comes free from a ones column appended to
    V (output row 64). DoubleRow weights are padded to 80 columns (LDW
    requires free dim % 16 == 0).
  - Host epilogue: O = (O^T_unnorm[:64] / l).T per head (the e^-c0
    factor cancels between O and l).
"""

import os
import sys
import numpy as np

sys.path.insert(0, "/opt/trn_rl_repo")

import concourse.bass as bass
import concourse.mybir as mybir
from concourse.tile import TileContext

B, H, S, D = 1, 16, 4096, 64
N_CORES = 8
H_PER = H // N_CORES          # heads per core
QB = 512                      # q-block (PSUM bank width in f32)
KT = 128                      # k-tile
NQB = S // QB                 # 8
NKT = S // KT                 # 32
VW = D + 1                    # V columns + ones column for the l sum
VW8 = 80                      # DoubleRow lhsT free dim must be 16-aligned
VWP = 128                     # PV weights padded to 128 cols for FWL

F32 = mybir.dt.float32
F16 = mybir.dt.float16
BF16 = mybir.dt.bfloat16
FP8 = mybir.dt.float8e4
I16 = mybir.dt.int16

MASKV = -480.0                # additive mask; exp(0.125*-480) ~ 9e-27
LN2 = float(np.log(2.0))


def cfg():
    dr = os.environ.get("ATTN_DR", "0") == "1"
    dve_bias = float(os.environ.get("ATTN_DVE_BIAS", "1.0"))
    return dr, dve_bias


def schraud_consts(c0: float):
    a16 = 0.125 * 1024.0 / LN2
    b16 = 15.0 * 1024.0 - 44.75 - c0 * 1024.0 / LN2
    return a16, b16


def build_program() -> bass.Bass:
    dr, dve_bias = cfg()
    c0 = 2.5 if dr else 0.0  # keep exp(s_max=8.09 - c0) < fp8e4m3 max 448
    a16, b16 = schraud_consts(c0)
    ovw = VW8 if dr else VWP

    nc = bass.Bass()
    qk_d = nc.declare_dram_parameter("qk", [H_PER, 64, 2 * S], F16,
                                     isOutput=False)
    v16_d = nc.declare_dram_parameter("v16", [H_PER, 128, NKT * VWP], F16,
                                      isOutput=False)
    if dr:
        v8_d = nc.declare_dram_parameter(
            "v8", [H_PER, 128, NKT // 2, 2, VW8], FP8, isOutput=False)
    mk_d = nc.declare_dram_parameter("mk", [128, 3 * KT], F16, isOutput=False)
    oT_d = nc.declare_dram_parameter("outT", [H_PER, VW, S], F32,
                                     isOutput=True)

    with TileContext(nc) as tc:
        with (
            tc.tile_pool(name="const", bufs=1) as cpool,
            tc.tile_pool(name="io", bufs=1) as iopool,
            tc.tile_pool(name="pa", bufs=3) as papool,
            tc.tile_pool(name="pd", bufs=3) as pdpool,
            tc.tile_pool(name="st", bufs=3, space="PSUM") as stpool,
            tc.tile_pool(name="ot", bufs=2, space="PSUM") as otpool,
        ):
            mks = cpool.tile([128, 3 * KT], F16, name="mks")
            nc.sync.dma_start(out=mks, in_=mk_d[:, :])
            bias_ap = 0.0
            if dr:
                biasT = cpool.tile([128, 1], F32, name="biasT")
                nc.vector.memset(biasT, -c0)
                bias_ap = biasT
            tri = mks[:, 0:KT]
            ident = mks[:, KT:2 * KT]
            full = mks[:, 2 * KT:3 * KT]

            # bf16 warmup matmuls keep the PE HAM busy during input DMA
            n_warm = int(os.environ.get("ATTN_WARM", "14"))
            if n_warm:
                wsrc = cpool.tile([128, QB], BF16, name="wsrc")
                nc.vector.memset(wsrc, 1.0)
                wps = otpool.tile([ovw, QB], F32, name="warmps", tag="otp")
                for _ in range(n_warm):
                    nc.tensor.matmul(
                        out=wps, lhsT=wsrc[:, 0:ovw], rhs=wsrc,
                        start=True, stop=True,
                    )

            head_ctx = []
            for h in range(H_PER):
                v16 = iopool.tile([128, NKT * VWP], F16, name=f"v16_{h}")
                v8 = None
                if dr:
                    v8 = iopool.tile([128, NKT // 2, 2, VW8], FP8,
                                     name=f"v8_{h}")
                qkts = iopool.tile([64, 2 * S], F16, name=f"qkts{h}")
                outs = iopool.tile([VW, S], F32, name=f"outs{h}")
                if h == 0:
                    # stage q-block 0's slices first so compute starts early
                    nc.sync.dma_start(out=v16[:, 0:4 * VWP],
                                      in_=v16_d[h][:, 0:4 * VWP])
                    if dr:
                        nc.sync.dma_start(out=v8[:, 0:2], in_=v8_d[h][:, 0:2])
                    nc.sync.dma_start(out=qkts[:, 0:QB], in_=qk_d[h][:, 0:QB])
                    nc.sync.dma_start(out=qkts[:, S:S + QB],
                                      in_=qk_d[h][:, S:S + QB])
                    nc.sync.dma_start(out=v16[:, 4 * VWP:],
                                      in_=v16_d[h][:, 4 * VWP:])
                    if dr:
                        nc.sync.dma_start(out=v8[:, 2:], in_=v8_d[h][:, 2:])
                    nc.sync.dma_start(out=qkts[:, QB:S], in_=qk_d[h][:, QB:S])
                    nc.sync.dma_start(out=qkts[:, S + QB:2 * S],
                                      in_=qk_d[h][:, S + QB:2 * S])
                else:
                    nc.sync.dma_start(out=v16, in_=v16_d[h])
                    if dr:
                        nc.sync.dma_start(out=v8, in_=v8_d[h][:, :, :, :])
                    nc.sync.dma_start(out=qkts[:, 0:S], in_=qk_d[h][:, 0:S])
                    nc.sync.dma_start(out=qkts[:, S:2 * S],
                                      in_=qk_d[h][:, S:2 * S])
                head_ctx.append((v16, v8, qkts, outs))

            # chunk list: (h, j, t0) pairs of k-tiles; diag iff t0 >= 4j
            all_chunks = []
            for h in range(H_PER):
                for j in range(NQB):
                    n_kt = 4 * (j + 1)
                    for t0 in range(0, n_kt, 2):
                        all_chunks.append((h, j, t0))

            # engine assignment: diag -> ACT; others balance modeled busy-ns
            eng_ns = {"act": 2700.0, "dve": 0.0}

            def exp_cost(engine, w2):
                # w2 = total free elements of the exp instruction
                if engine == "act":
                    return (w2 + 352.0) / 1.2
                return (w2 + 178.0) / 0.96

            assign = {}
            for ch in all_chunks:
                h, j, t0 = ch
                diag = t0 >= 4 * j
                ws = 128 * max(t0 - 4 * j, 0)
                w2 = 2 * (QB - ws)
                if diag or (dr and t0 == 0):
                    # t0==0 -> ACT/DR so start=True covers all 80 otp rows
                    e = "act"
                else:
                    ca = eng_ns["act"] + exp_cost("act", w2)
                    cd = eng_ns["dve"] + exp_cost("dve", w2) * dve_bias
                    e = "act" if ca <= cd else "dve"
                assign[ch] = e
                scale = dve_bias if e == "dve" else 1.0
                eng_ns[e] += exp_cost(e, w2) * scale

            otp_box = {}
            copy_flip = [0]

            def emit_front(chunk):
                """QK matmuls + masks + exp for one 2-k-tile chunk."""
                h, j, t0 = chunk
                v16, v8, qkts, outs = head_ctx[h]
                engine = assign[chunk]
                stp = stpool.tile([128, 2, QB], F32, name="stp", tag="stp")
                diag = t0 >= 4 * j
                ws = 128 * max(t0 - 4 * j, 0)      # union window start
                qs = j * QB
                for u in (0, 1):
                    ki = t0 + u
                    wsu = 128 * max(ki - 4 * j, 0)
                    nc.tensor.matmul(
                        out=stp[:, u, wsu:],
                        lhsT=qkts[:, S + ki * KT:S + (ki + 1) * KT],
                        rhs=qkts[:, qs + wsu:qs + QB],
                        start=True, stop=(not diag and u == 1),
                        skip_group_check=True,
                    )
                # triangle masks (adjacent -> shared LDWEIGHTS pattern)
                if diag:
                    for u in (0, 1):
                        tl = t0 + u - 4 * j
                        nc.tensor.matmul(
                            out=stp[:, u, 128 * tl:128 * (tl + 1)],
                            lhsT=tri, rhs=ident,
                            start=False, stop=(u == 1), skip_group_check=True,
                        )
                if engine == "act":
                    pdt = FP8 if dr else F16
                    pt = papool.tile([128, 2, QB], pdt, name="pa", tag="pa")
                    if diag and dr:
                        # per-plane windows; plane B's fully-masked block is
                        # neither exp'd nor consumed (DR would need it)
                        raise NotImplementedError("dr diag per-plane")
                    if diag:
                        for u in (0, 1):
                            wsu = 128 * (t0 + u - 4 * j)
                            nc.scalar.activation(
                                out=pt[:, u, wsu:], in_=stp[:, u, wsu:],
                                func=mybir.ActivationFunctionType.Exp,
                                scale=0.125, bias=bias_ap,
                            )
                    else:
                        nc.scalar.activation(
                            out=pt[:, :, ws:], in_=stp[:, :, ws:],
                            func=mybir.ActivationFunctionType.Exp,
                            scale=0.125, bias=bias_ap,
                        )
                else:
                    pt = pdpool.tile([128, 2, QB], I16, name="pd", tag="pd")
                    nc.vector.tensor_scalar(
                        pt[:, :, ws:], stp[:, :, ws:], a16, b16,
                        mybir.AluOpType.mult, mybir.AluOpType.add,
                    )
                return pt

            def emit_back(chunk, pt):
                """PV matmuls (+ output copy/DMA on the last chunk)."""
                h, j, t0 = chunk
                v16, v8, qkts, outs = head_ctx[h]
                engine = assign[chunk]
                n_kt = 4 * (j + 1)
                if (h, j) not in otp_box:
                    otp_box[(h, j)] = otpool.tile([ovw, QB], F32, name="otp",
                                                  tag="otp")
                otp = otp_box[(h, j)]
                ws = 128 * max(t0 - 4 * j, 0)
                if engine == "act" and dr:
                    nc.tensor.matmul(
                        out=otp[:, ws:], lhsT=v8[:, t0 // 2],
                        rhs=pt[:, :, ws:],
                        start=(t0 == 0), stop=(t0 + 2 == n_kt),
                        perf_mode=mybir.MatmulPerfMode.DoubleRow,
                        skip_group_check=True,
                    )
                else:
                    for u in (0, 1):
                        ki = t0 + u
                        wsu = 128 * max(ki - 4 * j, 0)
                        rhs = pt[:, u, wsu:]
                        if engine == "dve":
                            rhs = rhs.bitcast(F16)
                        nc.tensor.matmul(
                            out=otp[0:VWP, wsu:],
                            lhsT=v16[:, ki * VWP:(ki + 1) * VWP], rhs=rhs,
                            start=(ki == 0), stop=(ki == n_kt - 1),
                            skip_group_check=True,
                        )
                if t0 + 2 == n_kt:
                    # q-block done: copy PSUM -> SBUF, then DMA out
                    del otp_box[(h, j)]
                    if copy_flip[0] % 2 == 0:
                        nc.vector.tensor_copy(
                            out=outs[:, j * QB:(j + 1) * QB], in_=otp[0:VW, :])
                    else:
                        nc.scalar.copy(
                            out=outs[:, j * QB:(j + 1) * QB], in_=otp[0:VW, :])
                    copy_flip[0] += 1
                    nc.sync.dma_start(
                        out=oT_d[h][:, j * QB:(j + 1) * QB],
                        in_=outs[:, j * QB:(j + 1) * QB],
                    )

            pending = None
            for chunk in all_chunks:
                pt = emit_front(chunk)
                if pending is not None:
                    emit_back(*pending)
                pending = (chunk, pt)
            emit_back(*pending)

    import concourse.bacc as baccmod

    baccmod._bass_rust.generate_event_semaphores(nc)
    return nc


_PROGRAM_CACHE: dict[str, bass.Bass] = {}


def get_program() -> bass.Bass:
    key = repr(cfg())
    if key not in _PROGRAM_CACHE:
        _PROGRAM_CACHE[key] = build_program()
    return _PROGRAM_CACHE[key]


def make_masks() -> np.ndarray:
    # lhsT layouts [r, k]; identity rhs picks r = q, so M[k, q] = lhsT[q, k]:
    # tri masks q < k (within the 128-wide diagonal block); full masks all.
    rr = np.arange(KT)[:, None]
    kk = np.arange(KT)[None, :]
    mk = np.empty((128, 3, KT), dtype=np.float16)
    mk[:, 0, :] = np.where(rr < kk, MASKV, 0.0)
    mk[:, 1, :] = (rr == kk).astype(np.float16)
    mk[:, 2, :] = MASKV
    return np.ascontiguousarray(mk.reshape(128, 3 * KT))


def make_in_maps(q, k, v):
    import ml_dtypes

    dr, _ = cfg()
    q = np.asarray(q, dtype=np.float32)
    k = np.asarray(k, dtype=np.float32)
    v = np.asarray(v, dtype=np.float32)
    mk = make_masks()
    in_maps = []
    for c in range(N_CORES):
        hs = [H_PER * c + i for i in range(H_PER)]
        qk = np.empty((H_PER, 64, 2 * S), dtype=np.float16)
        v16 = np.zeros((H_PER, 128, NKT, VWP), dtype=np.float16)
        if dr:
            v8 = np.zeros((H_PER, 128, NKT // 2, 2, VW8),
                          dtype=ml_dtypes.float8_e4m3fn)
        for i, h in enumerate(hs):
            qk[i, :, 0:S] = q[0, h].T
            qk[i, :, S:2 * S] = k[0, h].T
            # [S, D] -> k-tiles on partitions: [128, NKT, D]
            vt = v[0, h].reshape(NKT, KT, D).transpose(1, 0, 2)
            v16[i, :, :, :D] = vt
            v16[i, :, :, D] = 1.0
            if dr:
                v8[i, :, :, 0, :D] = vt[:, 0::2]
                v8[i, :, :, 1, :D] = vt[:, 1::2]
                v8[i, :, :, 0, D] = 1.0
                v8[i, :, :, 1, D] = 1.0
        m = {"qk": qk,
             "v16": np.ascontiguousarray(v16.reshape(H_PER, 128, NKT * VWP)),
             "mk": mk}
        if dr:
            m["v8"] = np.ascontiguousarray(v8)
        in_maps.append(m)
    return in_maps


def assemble_output(results) -> np.ndarray:
    out = np.empty((B, H, S, D), dtype=np.float32)
    for c in range(N_CORES):
        oT = results[c]["outT"]  # [H_PER, VW, S]
        for i in range(H_PER):
            h = H_PER * c + i
            out[0, h] = (oT[i, :D, :] / oT[i, D:D + 1, :]).T
    return out


def run_sharded(q, k, v, trace: bool = False):
    from concourse.bass_utils import run_bass_kernel_spmd

    nc = get_program()
    in_maps = make_in_maps(q, k, v)
    res = run_bass_kernel_spmd(
        nc, in_maps, list(range(N_CORES)), trace=trace
    )
    return assemble_output(res.results), res


def kernel(q, k, v, mask=None) -> np.ndarray:
    # mask is deterministically the causal tril mask; causality is baked in.
    out, _ = run_sharded(q, k, v, trace=False)
    return out


# revision 13
# speedup vs baseline: 1.0315x; 1.0315x over previous
"""Causal attention (B=1, H=16, S=4096, D=64, f32) on 8 trn2 NeuronCores.

Strategy (head-parallel, 2 heads per core), v2:
  - Host pre-transposes Q, K per head to [D, S] fp16 (d-major) so QK^T
    needs no on-device transpose: S^T[k, q] = sum_d K^T[d,k] Q^T[d,q].
    One fp16 matmul per (k-tile, q-block): lhsT [64,128], rhs [64,512],
    ~217 ns (output-partition bound: 128 scores/cycle is the PE limit).
  - Causal masking is additive and done on the PE: tiny fp16 matmuls
    (~60 ns) accumulate -480 into masked score regions of PSUM (triangle
    / full-block lhsT against an identity rhs), so exp(0.125*s) is ~1e-26
    there and no vector-engine masking is needed.
  - exp is split across two engines, processing chunks of 2 k-tiles:
      * ScalarE ACTIVATE: exact exp, fp8e4 out with an e^-2.5 shift
        (keeps exp under fp8e4m3 max 448); consumed by fp8
        DoubleRow PV matmuls - 2 k-tiles per 512-cycle stream (ATTN_DR=1),
        or fp16 out + fp16 PV with ATTN_DR=0.
      * VectorE tensor_scalar: Schraudolph bit-trick exp: int16
        rint(A*x+B) whose bits read as fp16 give e^(0.125x-c0) to ~3%;
        consumed (bitcast) by fp16 PV matmuls. The f32->int16 conversion
        saturates, so -480-masked scores become -32768 = fp16 -0.0.
    Diagonal chunks always go to ScalarE: they hold the dominant
    self-attention weights (s(q,q) ~ +8 sigma), where 3% error is not
    acceptable.
  - PV: O^T[d, q] = sum_k V[k, d] P^T[k, q], PSUM-accumulated over
    k-tiles. l[q] = sum_k exp comes free from a ones column appended to
    V (output row 64). DoubleRow weights are padded to 80 columns (LDW
    requires free dim % 16 == 0).
  - Host epilogue: O = (O^T_unnorm[:64] / l).T per head (the e^-c0
    factor cancels between O and l).
"""

import os
import sys
import numpy as np

sys.path.insert(0, "/opt/trn_rl_repo")

import concourse.bass as bass
import concourse.mybir as mybir
from concourse.tile import TileContext

B, H, S, D = 1, 16, 4096, 64
N_CORES = 8
H_PER = H // N_CORES          # heads per core
QB = 512                      # q-block (PSUM bank width in f32)
KT = 128                      # k-tile
NQB = S // QB                 # 8
NKT = S // KT                 # 32
VW = D + 1                    # V columns + ones column for the l sum
VW8 = 80                      # DoubleRow lhsT free dim must be 16-aligned
VWP = 128                     # PV weights padded to 128 cols for FWL

F32 = mybir.dt.float32
F16 = mybir.dt.float16
BF16 = mybir.dt.bfloat16
FP8 = mybir.dt.float8e4
I16 = mybir.dt.int16

MASKV = -480.0                # additive mask; exp(0.125*-480) ~ 9e-27
LN2 = float(np.log(2.0))


def cfg():
    dr = os.environ.get("ATTN_DR", "0") == "1"
    dve_bias = float(os.environ.get("ATTN_DVE_BIAS", "1.0"))
    return dr, dve_bias


def schraud_consts(c0: float):
    a16 = 0.125 * 1024.0 / LN2
    b16 = 15.0 * 1024.0 - 44.75 - c0 * 1024.0 / LN2
    return a16, b16


def build_program() -> bass.Bass:
    dr, dve_bias = cfg()
    c0 = 2.5 if dr else 0.0  # keep exp(s_max=8.09 - c0) < fp8e4m3 max 448
    a16, b16 = schraud_consts(c0)
    ovw = VW8 if dr else VWP

    nc = bass.Bass()
    qk_d = nc.declare_dram_parameter("qk", [H_PER, 64, 2 * S], F16,
                                     isOutput=False)
    v16_d = nc.declare_dram_parameter("v16", [H_PER, 128, NKT * VWP], F16,
                                      isOutput=False)
    if dr:
        v8_d = nc.declare_dram_parameter(
            "v8", [H_PER, 128, NKT // 2, 2, VW8], FP8, isOutput=False)
    mk_d = nc.declare_dram_parameter("mk", [128, 3 * KT], F16, isOutput=False)
    oT_d = nc.declare_dram_parameter("outT", [H_PER, VW, S], F32,
                                     isOutput=True)

    with TileContext(nc) as tc:
        with (
            tc.tile_pool(name="const", bufs=1) as cpool,
            tc.tile_pool(name="io", bufs=1) as iopool,
            tc.tile_pool(name="pa", bufs=3) as papool,
            tc.tile_pool(name="pd", bufs=3) as pdpool,
            tc.tile_pool(name="st", bufs=3, space="PSUM") as stpool,
            tc.tile_pool(name="ot", bufs=2, space="PSUM") as otpool,
        ):
            mks = cpool.tile([128, 3 * KT], F16, name="mks")
            nc.sync.dma_start(out=mks, in_=mk_d[:, :])
            bias_ap = 0.0
            if dr:
                biasT = cpool.tile([128, 1], F32, name="biasT")
                nc.vector.memset(biasT, -c0)
                bias_ap = biasT
            tri = mks[:, 0:KT]
            ident = mks[:, KT:2 * KT]
            full = mks[:, 2 * KT:3 * KT]

            # bf16 warmup matmuls keep the PE HAM busy during input DMA
            n_warm = int(os.environ.get("ATTN_WARM", "14"))
            if n_warm:
                wsrc = cpool.tile([128, QB], BF16, name="wsrc")
                nc.vector.memset(wsrc, 1.0)
                wps = otpool.tile([ovw, QB], F32, name="warmps", tag="otp")
                for _ in range(n_warm):
                    nc.tensor.matmul(
                        out=wps, lhsT=wsrc[:, 0:ovw], rhs=wsrc,
                        start=True, stop=True,
                    )

            head_ctx = []
            for h in range(H_PER):
                v16 = iopool.tile([128, NKT * VWP], F16, name=f"v16_{h}")
                v8 = None
                if dr:
                    v8 = iopool.tile([128, NKT // 2, 2, VW8], FP8,
                                     name=f"v8_{h}")
                qkts = iopool.tile([64, 2 * S], F16, name=f"qkts{h}")
                outs = iopool.tile([VW, S], F32, name=f"outs{h}")
                if h == 0:
                    # stage q-block 0's slices first so compute starts early
                    nc.sync.dma_start(out=v16[:, 0:4 * VWP],
                                      in_=v16_d[h][:, 0:4 * VWP])
                    if dr:
                        nc.sync.dma_start(out=v8[:, 0:2], in_=v8_d[h][:, 0:2])
                    nc.sync.dma_start(out=qkts[:, 0:QB], in_=qk_d[h][:, 0:QB])
                    nc.sync.dma_start(out=qkts[:, S:S + QB],
                                      in_=qk_d[h][:, S:S + QB])
                    nc.sync.dma_start(out=v16[:, 4 * VWP:],
                                      in_=v16_d[h][:, 4 * VWP:])
                    if dr:
                        nc.sync.dma_start(out=v8[:, 2:], in_=v8_d[h][:, 2:])
                    nc.sync.dma_start(out=qkts[:, QB:S], in_=qk_d[h][:, QB:S])
                    nc.sync.dma_start(out=qkts[:, S + QB:2 * S],
                                      in_=qk_d[h][:, S + QB:2 * S])
                else:
                    nc.sync.dma_start(out=v16, in_=v16_d[h])
                    if dr:
                        nc.sync.dma_start(out=v8, in_=v8_d[h][:, :, :, :])
                    nc.sync.dma_start(out=qkts[:, 0:S], in_=qk_d[h][:, 0:S])
                    nc.sync.dma_start(out=qkts[:, S:2 * S],
                                      in_=qk_d[h][:, S:2 * S])
                head_ctx.append((v16, v8, qkts, outs))

            # chunk list: (h, j, t0) pairs of k-tiles; diag iff t0 >= 4j
            all_chunks = []
            for h in range(H_PER):
                for j in range(NQB):
                    n_kt = 4 * (j + 1)
                    for t0 in range(0, n_kt, 2):
                        all_chunks.append((h, j, t0))

            # engine assignment: diag -> ACT; others balance modeled busy-ns
            eng_ns = {"act": 2700.0, "dve": 0.0}

            def exp_cost(engine, w2):
                # w2 = total free elements of the exp instruction
                if engine == "act":
                    return (w2 + 352.0) / 1.2
                return (w2 + 178.0) / 0.96

            assign = {}
            for ch in all_chunks:
                h, j, t0 = ch
                diag = t0 >= 4 * j
                ws = 128 * max(t0 - 4 * j, 0)
                w2 = 2 * (QB - ws)
                if diag or (dr and t0 == 0):
                    # t0==0 -> ACT/DR so start=True covers all 80 otp rows
                    e = "act"
                else:
                    ca = eng_ns["act"] + exp_cost("act", w2)
                    cd = eng_ns["dve"] + exp_cost("dve", w2) * dve_bias
                    e = "act" if ca <= cd else "dve"
                assign[ch] = e
                scale = dve_bias if e == "dve" else 1.0
                eng_ns[e] += exp_cost(e, w2) * scale

            otp_box = {}
            copy_flip = [0]

            def emit_front(chunk):
                """QK matmuls + masks + exp for one 2-k-tile chunk."""
                h, j, t0 = chunk
                v16, v8, qkts, outs = head_ctx[h]
                engine = assign[chunk]
                stp = stpool.tile([128, 2, QB], F32, name="stp", tag="stp")
                diag = t0 >= 4 * j
                ws = 128 * max(t0 - 4 * j, 0)      # union window start
                qs = j * QB
                for u in (0, 1):
                    ki = t0 + u
                    wsu = 128 * max(ki - 4 * j, 0)
                    nc.tensor.matmul(
                        out=stp[:, u, wsu:],
                        lhsT=qkts[:, S + ki * KT:S + (ki + 1) * KT],
                        rhs=qkts[:, qs + wsu:qs + QB],
                        start=True, stop=(not diag and u == 1),
                        skip_group_check=True,
                    )
                # triangle masks (adjacent -> shared LDWEIGHTS pattern)
                if diag:
                    for u in (0, 1):
                        tl = t0 + u - 4 * j
                        nc.tensor.matmul(
                            out=stp[:, u, 128 * tl:128 * (tl + 1)],
                            lhsT=tri, rhs=ident,
                            start=False, stop=(u == 1), skip_group_check=True,
                        )
                if engine == "act":
                    pdt = FP8 if dr else F16
                    pt = papool.tile([128, 2, QB], pdt, name="pa", tag="pa")
                    if diag and dr:
                        # per-plane windows; plane B's fully-masked block is
                        # neither exp'd nor consumed (DR would need it)
                        raise NotImplementedError("dr diag per-plane")
                    if diag:
                        for u in (0, 1):
                            wsu = 128 * (t0 + u - 4 * j)
                            nc.scalar.activation(
                                out=pt[:, u, wsu:], in_=stp[:, u, wsu:],
                                func=mybir.ActivationFunctionType.Exp,
                                scale=0.125, bias=bias_ap,
                            )
                    else:
                        nc.scalar.activation(
                            out=pt[:, :, ws:], in_=stp[:, :, ws:],
                            func=mybir.ActivationFunctionType.Exp,
                            scale=0.125, bias=bias_ap,
                        )
                else:
                    pt = pdpool.tile([128, 2, QB], I16, name="pd", tag="pd")
                    nc.vector.tensor_scalar(
                        pt[:, :, ws:], stp[:, :, ws:], a16, b16,
                        mybir.AluOpType.mult, mybir.AluOpType.add,
                    )
                return pt

            def emit_back(chunk, pt):
                """PV matmuls (+ output copy/DMA on the last chunk)."""
                h, j, t0 = chunk
                v16, v8, qkts, outs = head_ctx[h]
                engine = assign[chunk]
                n_kt = 4 * (j + 1)
                if (h, j) not in otp_box:
                    otp_box[(h, j)] = otpool.tile([ovw, QB], F32, name="otp",
                                                  tag="otp")
                otp = otp_box[(h, j)]
                ws = 128 * max(t0 - 4 * j, 0)
                if engine == "act" and dr:
                    nc.tensor.matmul(
                        out=otp[:, ws:], lhsT=v8[:, t0 // 2],
                        rhs=pt[:, :, ws:],
                        start=(t0 == 0), stop=(t0 + 2 == n_kt),
                        perf_mode=mybir.MatmulPerfMode.DoubleRow,
                        skip_group_check=True,
                    )
                else:
                    for u in (0, 1):
                        ki = t0 + u
                        wsu = 128 * max(ki - 4 * j, 0)
                        rhs = pt[:, u, wsu:]
                        if engine == "dve":
                            rhs = rhs.bitcast(F16)
                        nc.tensor.matmul(
                            out=otp[0:VWP, wsu:],
                            lhsT=v16[:, ki * VWP:(ki + 1) * VWP], rhs=rhs,
                            start=(ki == 0), stop=(ki == n_kt - 1),
                            skip_group_check=True,
                        )
                if t0 + 2 == n_kt:
                    # q-block done: copy PSUM -> SBUF, then DMA out
                    del otp_box[(h, j)]
                    if copy_flip[0] % 2 == 0:
                        nc.vector.tensor_copy(
                            out=outs[:, j * QB:(j + 1) * QB], in_=otp[0:VW, :])
                    else:
                        nc.scalar.copy(
                            out=outs[:, j * QB:(j + 1) * QB], in_=otp[0:VW, :])
                    copy_flip[0] += 1
                    nc.sync.dma_start(
                        out=oT_d[h][:, j * QB:(j + 1) * QB],
                        in_=outs[:, j * QB:(j + 1) * QB],
                    )

            # 2-deep software pipeline: PV matmuls trail their chunk's
            # exp by two chunks so the matmul's semaphore wait is already
            # satisfied when it reaches the PE queue (an unsatisfied wait
            # blocks the LDWEIGHTS slot and costs ~110 ns per matmul).
            lag = int(os.environ.get("ATTN_LAG", "2"))
            from collections import deque
            pending = deque()
            for chunk in all_chunks:
                pt = emit_front(chunk)
                pending.append((chunk, pt))
                if len(pending) > lag:
                    emit_back(*pending.popleft())
            while pending:
                emit_back(*pending.popleft())

    import concourse.bacc as baccmod

    baccmod._bass_rust.generate_event_semaphores(nc)
    return nc


_PROGRAM_CACHE: dict[str, bass.Bass] = {}


def get_program() -> bass.Bass:
    key = repr(cfg())
    if key not in _PROGRAM_CACHE:
        _PROGRAM_CACHE[key] = build_program()
    return _PROGRAM_CACHE[key]


def make_masks() -> np.ndarray:
    # lhsT layouts [r, k]; identity rhs picks r = q, so M[k, q] = lhsT[q, k]:
    # tri masks q < k (within the 128-wide diagonal block); full masks all.
    rr = np.arange(KT)[:, None]
    kk = np.arange(KT)[None, :]
    mk = np.empty((128, 3, KT), dtype=np.float16)
    mk[:, 0, :] = np.where(rr < kk, MASKV, 0.0)
    mk[:, 1, :] = (rr == kk).astype(np.float16)
    mk[:, 2, :] = MASKV
    return np.ascontiguousarray(mk.reshape(128, 3 * KT))


def make_in_maps(q, k, v):
    import ml_dtypes

    dr, _ = cfg()
    q = np.asarray(q, dtype=np.float32)
    k = np.asarray(k, dtype=np.float32)
    v = np.asarray(v, dtype=np.float32)
    mk = make_masks()
    in_maps = []
    for c in range(N_CORES):
        hs = [H_PER * c + i for i in range(H_PER)]
        qk = np.empty((H_PER, 64, 2 * S), dtype=np.float16)
        v16 = np.zeros((H_PER, 128, NKT, VWP), dtype=np.float16)
        if dr:
            v8 = np.zeros((H_PER, 128, NKT // 2, 2, VW8),
                          dtype=ml_dtypes.float8_e4m3fn)
        for i, h in enumerate(hs):
            qk[i, :, 0:S] = q[0, h].T
            qk[i, :, S:2 * S] = k[0, h].T
            # [S, D] -> k-tiles on partitions: [128, NKT, D]
            vt = v[0, h].reshape(NKT, KT, D).transpose(1, 0, 2)
            v16[i, :, :, :D] = vt
            v16[i, :, :, D] = 1.0
            if dr:
                v8[i, :, :, 0, :D] = vt[:, 0::2]
                v8[i, :, :, 1, :D] = vt[:, 1::2]
                v8[i, :, :, 0, D] = 1.0
                v8[i, :, :, 1, D] = 1.0
        m = {"qk": qk,
             "v16": np.ascontiguousarray(v16.reshape(H_PER, 128, NKT * VWP)),
             "mk": mk}
        if dr:
            m["v8"] = np.ascontiguousarray(v8)
        in_maps.append(m)
    return in_maps


def assemble_output(results) -> np.ndarray:
    out = np.empty((B, H, S, D), dtype=np.float32)
    for c in range(N_CORES):
        oT = results[c]["outT"]  # [H_PER, VW, S]
        for i in range(H_PER):
            h = H_PER * c + i
            out[0, h] = (oT[i, :D, :] / oT[i, D:D + 1, :]).T
    return out


def run_sharded(q, k, v, trace: bool = False):
    from concourse.bass_utils import run_bass_kernel_spmd

    nc = get_program()
    in_maps = make_in_maps(q, k, v)
    res = run_bass_kernel_spmd(
        nc, in_maps, list(range(N_CORES)), trace=trace
    )
    return assemble_output(res.results), res


def kernel(q, k, v, mask=None) -> np.ndarray:
    # mask is deterministically the causal tril mask; causality is baked in.
    out, _ = run_sharded(q, k, v, trace=False)
    return out
